# revision 1
# baseline (speedup 1.0000x reference)
"""Multi-level dense 3D conv (AbstractConv3D) as a Trainium2 Bass kernel.

v2: tap-pairing. The 27-tap stencil is computed as 9 paired matmuls
(32-deep contraction: taps (dx,dy,-1)+(dx,dy,0) share one matmul whose
rhs rows 16..31 hold a copy of x shifted by one token) plus 9 single
matmuls (taps (dx,dy,+1), 16-deep). 18 matmuls/window instead of 27.
Bias is added on the host during unpack (frees the ones-row).

Layout per 32-partition group q (one token-range chunk):
  rows 32q+0..16:  x[a + i]        (channel-major bf16)
  rows 32q+16..32: x[a + 1 + i]    (same data shifted one token)
Both blocks are DMA'd straight from DRAM (second read offset by one
element), so no on-chip shuffling is needed.
"""

import math
from contextlib import ExitStack

import numpy as np
import ml_dtypes

import concourse.bass as bass
import concourse.bacc as bacc
import concourse.mybir as mybir
import concourse.tile as tile
from concourse.bass_utils import run_bass_kernel_spmd

BF16 = ml_dtypes.bfloat16

RES = [16, 18, 20, 23, 26, 29, 32, 36, 40, 45, 50, 56, 63, 70, 76, 80]
L = 16
CIN = 16
COUT = 16
NCORES = 8
NWIN = 512  # matmul free dim / PSUM bank (f32)

# Per-level geometry
S_L = [math.ceil(r / 8) for r in RES]  # x-slabs per core
P_L = [r + 1 for r in RES]  # padded y/z extent
T_L = [(s + 2) * p * p for s, p in zip(S_L, P_L)]  # piece tokens (w/ x-halo)
H_L = [p * p + p + 1 for p in P_L]  # max |tap shift|
GUARD = 6656  # > max(H_L) + 1 = 6644
GAP = 256  # zero gap between levels: valid outputs read up to P+1 before level start
T_IN = GUARD + sum(T_L) + (L - 1) * GAP + GUARD
T_OUT = sum(T_L)
LVL_IN_BASE = [GUARD + sum(T_L[:i]) + i * GAP for i in range(L)]
LVL_OUT_BASE = [sum(T_L[:i]) for i in range(L)]

# Tap order (dx, dy, dz) row-major matching weight.reshape(27, CIN, COUT).
# Pair m (m<9): taps (dx,dy,-1)+(dx,dy,0) fused, 32-deep contraction.
# Single m (m>=9): tap (dx,dy,+1), 16-deep.
DXY = [(dx, dy) for dx in (-1, 0, 1) for dy in (-1, 0, 1)]
NMM = 18

_CACHE = {}


def _tap_index(dx, dy, dz):
    return (dx + 1) * 9 + (dy + 1) * 3 + (dz + 1)


def _build_program(levels=None, reps=1):
    """One SPMD program, identical for all cores."""
    if levels is None:
        levels = range(L)
    nc = bacc.Bacc("TRN2", target_bir_lowering=False, debug=False, num_devices=NCORES)
    x_ext = nc.declare_dram_parameter("x", [16, T_IN], mybir.dt.bfloat16, isOutput=False)
    w_ext = nc.declare_dram_parameter("w", [128, L * NMM * 16], mybir.dt.bfloat16, isOutput=False)
    out_ext = nc.declare_dram_parameter("o", [16, T_OUT], mybir.dt.bfloat16, isOutput=True)

    with tile.TileContext(nc) as tc, ExitStack() as ctx:
        w_pool = ctx.enter_context(tc.tile_pool(name="w", bufs=1))
        x_pool = ctx.enter_context(tc.tile_pool(name="x", bufs=2))
        ps_pool = ctx.enter_context(tc.tile_pool(name="ps", bufs=2, space="PSUM"))
        st_pool = ctx.enter_context(tc.tile_pool(name="st", bufs=3))

        w_sb = w_pool.tile([128, L * NMM * 16], mybir.dt.bfloat16)
        nc.sync.dma_start(w_sb[:], w_ext[:])

        for _rep in range(reps):
            _emit_body(nc, tc, x_pool, ps_pool, st_pool, w_sb, x_ext, out_ext, list(levels))
    nc.finalize()
    return nc


def _emit_body(nc, tc, x_pool, ps_pool, st_pool, w_sb, x_ext, out_ext, levels):
    for lvl in levels:
        P = P_L[lvl]
        T = T_L[lvl]
        H = H_L[lvl]
        # Compute only the real slabs [P^2, T-P^2): the x-halo slabs are
        # input-only (their "outputs" would be discarded by the host).
        TC = T - 2 * P * P
        Q = math.ceil(TC / 4)  # computed tokens per chunk (last may be smaller)
        qlens = [min(Q, TC - q * Q) for q in range(4)]
        aq = [P * P + q * Q for q in range(4)]  # chunk output start (level coords)
        F = Q + 2 * H + 1  # loaded extent per chunk (incl. +1 shifted block)
        nwin = math.ceil(Q / NWIN)

        xt = x_pool.tile([128, F], mybir.dt.bfloat16, tag="xchunk")
        SEG = 32768  # DMA descriptor rows must stay under 64KB
        for q in range(4):
            a = LVL_IN_BASE[lvl] + aq[q] - H
            fq = qlens[q] + 2 * H
            for s0 in range(0, fq, SEG):
                sl = min(SEG, fq - s0)
                nc.sync.dma_start(
                    xt[32 * q : 32 * q + 16, s0 : s0 + sl],
                    x_ext[:, a + s0 : a + s0 + sl],
                )
                nc.sync.dma_start(
                    xt[32 * q + 16 : 32 * q + 32, s0 : s0 + sl],
                    x_ext[:, a + 1 + s0 : a + 1 + s0 + sl],
                )

        # Free-dim offset of the first tap of each matmul.
        # Pairs anchor at (dx,dy,-1); singles at (dx,dy,+1).
        dd_pair = [dx * P * P + dy * P - 1 for dx, dy in DXY]
        dd_sing = [dx * P * P + dy * P + 1 for dx, dy in DXY]

        for t in range(math.ceil(nwin / 4)):
            ps = ps_pool.tile([128, 4 * NWIN], mybir.dt.float32)
            # Diagonal order: consecutive matmuls differ in BOTH row group q
            # (so LDWEIGHTS can be pulled ahead of in-flight matmuls) and
            # column group j (so the per-column-group XBUS streams overlap).
            live = []  # (q, j, w0, nw)
            for d in range(4):
                for q in range(4):
                    j = (q + d) % 4
                    w = t * 4 + j
                    w0 = H + w * NWIN
                    nw = min(NWIN, qlens[q] + H - w0)
                    if nw > 0:
                        live.append((q, j, w0, nw))
            for m in range(NMM):
                kk = 32 if m < 9 else 16
                dd = dd_pair[m] if m < 9 else dd_sing[m - 9]
                for q, j, w0, nw in live:
                    lhsT = w_sb[32 * q : 32 * q + kk, (lvl * NMM + m) * 16 : (lvl * NMM + m) * 16 + 16]
                    rhs = xt[32 * q : 32 * q + kk, w0 + dd : w0 + dd + nw]
                    nc.tensor.matmul(
                        ps[32 * j : 32 * j + 16, q * NWIN : q * NWIN + nw],
                        lhsT,
                        rhs,
                        start=(m == 0),
                        stop=(m == NMM - 1),
                        tile_position=(32 * q, 32 * j),
                    )
            st = st_pool.tile([128, 4 * NWIN], mybir.dt.bfloat16, tag="stage")
            half = 2 * NWIN
            nc.scalar.copy(st[:, 0:half], ps[:, 0:half])
            nc.vector.tensor_copy(st[:, half : 4 * NWIN], ps[:, half : 4 * NWIN])
            for q, j, w0, nw in live:
                ob = LVL_OUT_BASE[lvl] + aq[q] + (w0 - H)
                nc.sync.dma_start(
                    out_ext[:, ob : ob + nw],
                    st[32 * j : 32 * j + 16, q * NWIN : q * NWIN + nw],
                )


def _pack_inputs(input, weight, bias):
    """Host-side pad/cast/transpose/shard. Returns per-core in_maps."""
    x = np.asarray(input)[0]  # [N, 16] f32
    wt = np.asarray(weight).reshape(L, 27, CIN, COUT)

    # Weights: [128, L*18*16] bf16. Slot m<9: rows 32q+0..16 = W[l,(dx,dy,-1)],
    # rows 32q+16..32 = W[l,(dx,dy,0)]. Slot m>=9: rows 32q+0..16 = W[l,(dx,dy,1)].
    wb = np.zeros((128, L * NMM * 16), dtype=BF16)
    wrow = np.zeros((32, L, NMM, COUT), dtype=np.float32)
    for m, (dx, dy) in enumerate(DXY):
        wrow[0:16, :, m, :] = wt[:, _tap_index(dx, dy, -1)].transpose(1, 0, 2)
        wrow[16:32, :, m, :] = wt[:, _tap_index(dx, dy, 0)].transpose(1, 0, 2)
        wrow[0:16, :, 9 + m, :] = wt[:, _tap_index(dx, dy, 1)].transpose(1, 0, 2)
    wrow = wrow.reshape(32, L * NMM * COUT).astype(BF16)
    for q in range(4):
        wb[32 * q : 32 * q + 32, :] = wrow

    xs = [np.zeros((16, T_IN), dtype=BF16) for _ in range(NCORES)]

    off = 0
    for lvl, r in enumerate(RES):
        P, s = P_L[lvl], S_L[lvl]
        g = x[off : off + r**3].reshape(r, r, r, CIN)
        off += r**3
        gp = np.zeros((CIN, 8 * s + 2, P, P), dtype=BF16)
        gp[:, 1 : r + 1, 0:r, 0:r] = g.transpose(3, 0, 1, 2)
        for i in range(NCORES):
            piece = gp[:, i * s : i * s + s + 2].reshape(CIN, T_L[lvl])
            xs[i][:, LVL_IN_BASE[lvl] : LVL_IN_BASE[lvl] + T_L[lvl]] = piece

    return [{"x": xs[i], "w": wb} for i in range(NCORES)]


def _unpack_outputs(results, bias):
    """Assemble [1, N, 16] f32 from per-core padded channel-major outputs."""
    bs = np.asarray(bias, dtype=np.float32)
    n_total = sum(r**3 for r in RES)
    out = np.empty((1, n_total, CIN), dtype=np.float32)
    off = 0
    for lvl, r in enumerate(RES):
        P, s = P_L[lvl], S_L[lvl]
        for i in range(NCORES):
            n_i = min(s, r - i * s)
            if n_i <= 0:
                continue
            o = np.asarray(results[i]["o"], dtype=np.float32)
            piece = o[:, LVL_OUT_BASE[lvl] : LVL_OUT_BASE[lvl] + T_L[lvl]]
            piece = piece.reshape(CIN, s + 2, P, P)[:, 1 : 1 + n_i, 0:r, 0:r]
            dst = off + i * s * r * r
            out[0, dst : dst + n_i * r * r] = (
                piece.transpose(1, 2, 3, 0).reshape(-1, CIN) + bs[lvl]
            )
        off += r**3
    return out


def run(input, offsets, resolutions, weight, bias, trace=False, levels=None, **trace_kw):
    key = ("nc", tuple(levels) if levels is not None else None)
    if key not in _CACHE:
        _CACHE[key] = _build_program(levels)
    nc = _CACHE[key]
    in_maps = _pack_inputs(input, weight, bias)
    res = run_bass_kernel_spmd(nc, in_maps, list(range(NCORES)), trace=trace, **trace_kw)
    return _unpack_outputs(res.results, bias), res


def kernel(input, offsets, resolutions, weight, bias):
    out, _ = run(input, offsets, resolutions, weight, bias)
    return out



# revision 5
# speedup vs baseline: 1.8736x; 1.8736x over previous
"""Multi-level dense 3D conv (AbstractConv3D) as a Trainium2 Bass kernel.

v3: deep-contraction + dual-output-group + host realign.

Per level, the token axis is split into chunks processed in PAIRS on the
two 64-row PE strips (alternating strips keeps the ~34ns matmul cadence).
Each strip holds THREE z-shifted copies of the chunk's tokens (rows
16k+i = x[base + u + k], k=0..2), loaded by ONE overlapping-stride DMA
from HBM. A 512-token window is computed by 6 matmuls (48-deep, 32-wide
stationary): for each dx, matmul 'a' covers taps (dx,-1,dz) in the main
output group and (dx,0,dz) in the aux group (realigned by +P on the
host), matmul 'b' covers (dx,+1,dz) in main. Outputs are dumped raw
[128, OW] to DRAM; the host adds main+aux (and bias) during unpack.
"""

import math
from contextlib import ExitStack

import numpy as np
import ml_dtypes

import concourse.bass as bass
import concourse.bacc as bacc
import concourse.mybir as mybir
import concourse.tile as tile
from concourse.bass_utils import run_bass_kernel_spmd

BF16 = ml_dtypes.bfloat16

RES = [16, 18, 20, 23, 26, 29, 32, 36, 40, 45, 50, 56, 63, 70, 76, 80]
L = 16
CIN = 16
COUT = 16
NCORES = 8
NWIN = 512

# Per-level geometry
S_L = [math.ceil(r / 8) for r in RES]  # x-slabs per core
P_L = [r + 1 for r in RES]  # padded y/z extent
T_L = [(s + 2) * p * p for s, p in zip(S_L, P_L)]  # piece tokens (w/ x-halo)
TC_L = [t - 2 * p * p for t, p in zip(T_L, P_L)]  # computed tokens
GUARD = 1024
GAP = 1024  # zero gap between levels (reads stay within +-~700 of level)
T_IN = GUARD + sum(T_L) + (L - 1) * GAP + GUARD
LVL_IN_BASE = [GUARD + sum(T_L[:i]) + i * GAP for i in range(L)]

_CACHE = {}


def _tap_index(dx, dy, dz):
    return (dx + 1) * 9 + (dy + 1) * 3 + (dz + 1)


def _level_plan(lvl):
    """Chunking plan: nch chunks (even), qlen tokens each, nwin windows."""
    P = P_L[lvl]
    TC = TC_L[lvl]

    def geom(nch):
        qlen = math.ceil(TC / nch)
        nwin = (qlen - 1 + P) // NWIN + 1
        E = NWIN * (nwin - 1) + 2 * P * P + 2 * P + 529
        return qlen, nwin, E

    nch = 2
    while geom(nch)[2] > 40000:
        nch += 2
    qlen, nwin, E = geom(nch)
    return nch, qlen, nwin, E


PLAN = [_level_plan(l) for l in range(L)]


def _dump_layout():
    """Assign output-dump column ranges per (level, pair, psum-group).

    Window w of chunk c (pair-local strip ci): group g=w//8, bank b=(w%8)//2,
    row strip j=2*ci+(w%2). Dump col = groupbase + 512*b + u.
    Returns per-level list of pair layouts and total dump width.
    """
    col = 0
    levels = []
    for lvl in range(L):
        nch, qlen, nwin, E = PLAN[lvl]
        pairs = []
        for p in range(nch // 2):
            groups = []
            w = 0
            while w < nwin:
                nb = min(4, (nwin - w + 1) // 2)  # banks used this group
                groups.append((col, nb))
                col += 512 * nb
                w += 2 * nb
            pairs.append(groups)
        levels.append(pairs)
    return levels, col


DUMP, OW = _dump_layout()


def _build_program(levels=None):
    if levels is None:
        levels = range(L)
    nc = bacc.Bacc("TRN2", target_bir_lowering=False, debug=False, num_devices=NCORES)
    x_ext = nc.declare_dram_parameter("x", [16, T_IN], mybir.dt.bfloat16, isOutput=False)
    w_ext = nc.declare_dram_parameter("w", [128, L * 6 * 32], mybir.dt.bfloat16, isOutput=False)
    o_ext = nc.declare_dram_parameter("o", [128, OW], mybir.dt.bfloat16, isOutput=True)

    with tile.TileContext(nc) as tc, ExitStack() as ctx:
        w_pool = ctx.enter_context(tc.tile_pool(name="w", bufs=1))
        x_pool = ctx.enter_context(tc.tile_pool(name="x", bufs=2))
        ps_pool = ctx.enter_context(tc.tile_pool(name="ps", bufs=2, space="PSUM"))
        st_pool = ctx.enter_context(tc.tile_pool(name="st", bufs=2))

        w_sb = w_pool.tile([128, L * 6 * 32], mybir.dt.bfloat16)
        nc.sync.dma_start(w_sb[:], w_ext[:])

        for lvl in levels:
            _emit_level(nc, tc, x_pool, ps_pool, st_pool, w_sb, x_ext, o_ext, lvl)
    nc.finalize()
    return nc


def _emit_level(nc, tc, x_pool, ps_pool, st_pool, w_sb, x_ext, o_ext, lvl):
    P = P_L[lvl]
    nch, qlen, nwin, E = PLAN[lvl]
    PP = P * P

    for pair in range(nch // 2):
        xt = x_pool.tile([128, E], mybir.dt.bfloat16, tag="xchunk")
        # Load both chunks' 3 shifted copies: one overlapping-stride DMA each.
        for ci in range(2):
            c = 2 * pair + ci
            # chunk computed range starts at level token P*P + c*qlen
            B0 = LVL_IN_BASE[lvl] + PP + c * qlen - (PP + P + 2)
            SEG = 32768
            for s0 in range(0, E, SEG):
                sl = min(SEG, E - s0)
                for k in range(3):
                    nc.sync.dma_start(
                        xt[64 * ci + 16 * k : 64 * ci + 16 * k + 16, s0 : s0 + sl],
                        x_ext[:, B0 + k + s0 : B0 + k + s0 + sl],
                    )

        ngroups = math.ceil(nwin / 8)
        groups = DUMP[lvl][pair]
        for g in range(ngroups):
            gcol, nb = groups[g]
            ps = ps_pool.tile([128, 2048], mybir.dt.float32, tag="ps")
            for b in range(nb):
                for widx in range(2):
                    w = 8 * g + 2 * b + widx
                    if w >= nwin:
                        continue
                    for m in range(6):
                        kind, dxi = divmod(m, 3)
                        dx = dxi - 1
                        slot = lvl * 6 + kind * 3 + dxi
                        if kind == 0:
                            off = NWIN * w + dx * PP + PP + 1
                        else:
                            off = NWIN * w + dx * PP + PP + 2 * P + 1
                        for ci in range(2):
                            j = 2 * ci + widx
                            nc.tensor.matmul(
                                ps[32 * j : 32 * j + 32, 512 * b : 512 * b + 512],
                                w_sb[64 * ci : 64 * ci + 48, slot * 32 : slot * 32 + 32],
                                xt[64 * ci : 64 * ci + 48, off : off + 512],
                                start=(m == 0),
                                stop=(m == 5),
                                tile_position=(64 * ci, 32 * j),
                            )
            st = st_pool.tile([128, 2048], mybir.dt.bfloat16, tag="stage")
            half = 256 * nb
            nc.scalar.copy(st[:, 0:half], ps[:, 0:half])
            nc.vector.tensor_copy(st[:, half : 512 * nb], ps[:, half : 512 * nb])
            nc.sync.dma_start(o_ext[:, gcol : gcol + 512 * nb], st[:, 0 : 512 * nb])


def _pack_inputs(input, weight):
    """Host-side pad/cast/transpose/shard. Returns per-core in_maps."""
    x = np.asarray(input)[0]  # [N, 16] f32
    wt = np.asarray(weight).reshape(L, 27, CIN, COUT)

    wb = np.zeros((128, L * 6 * 32), dtype=np.float32)
    for lvl in range(L):
        for kind in range(2):
            for dxi, dx in enumerate((-1, 0, 1)):
                slot = lvl * 6 + kind * 3 + dxi
                S = np.zeros((48, 32), dtype=np.float32)
                for k in range(3):
                    if kind == 0:
                        S[16 * k : 16 * k + 16, 0:16] = wt[lvl, _tap_index(dx, -1, k - 1)]
                        S[16 * k : 16 * k + 16, 16:32] = wt[lvl, _tap_index(dx, 0, k - 1)]
                    else:
                        S[16 * k : 16 * k + 16, 0:16] = wt[lvl, _tap_index(dx, 1, k - 1)]
                wb[0:48, slot * 32 : slot * 32 + 32] = S
                wb[64:112, slot * 32 : slot * 32 + 32] = S
    wb = wb.astype(BF16)

    xs = [np.zeros((16, T_IN), dtype=BF16) for _ in range(NCORES)]
    off = 0
    for lvl, r in enumerate(RES):
        P, s = P_L[lvl], S_L[lvl]
        g = x[off : off + r**3].reshape(r, r, r, CIN)
        off += r**3
        gp = np.zeros((CIN, 8 * s + 2, P, P), dtype=BF16)
        gp[:, 1 : r + 1, 0:r, 0:r] = g.transpose(3, 0, 1, 2)
        for i in range(NCORES):
            piece = gp[:, i * s : i * s + s + 2].reshape(CIN, T_L[lvl])
            xs[i][:, LVL_IN_BASE[lvl] : LVL_IN_BASE[lvl] + T_L[lvl]] = piece

    return [{"x": xs[i], "w": wb} for i in range(NCORES)]


def _unpack_outputs(results, bias, levels=None):
    """Assemble [1, N, 16] f32 from per-core raw dumps (main+aux+bias)."""
    if levels is None:
        levels = range(L)
    bs = np.asarray(bias, dtype=np.float32)
    n_total = sum(r**3 for r in RES)
    out = np.zeros((1, n_total, CIN), dtype=np.float32)
    lvl_out_off = np.concatenate([[0], np.cumsum([r**3 for r in RES])])
    for i in range(NCORES):
        o = np.asarray(results[i]["o"], dtype=np.float32)  # [128, OW]
        for lvl in levels:
            r = RES[lvl]
            P, s = P_L[lvl], S_L[lvl]
            nch, qlen, nwin, E = PLAN[lvl]
            TC = TC_L[lvl]
            n_i = min(s, r - i * s)
            if n_i <= 0:
                continue
            toks = np.zeros((16, TC), dtype=np.float32)
            for pair in range(nch // 2):
                for ci in range(2):
                    c = 2 * pair + ci
                    c0 = c * qlen
                    if c0 >= TC:
                        continue
                    ql = min(qlen, TC - c0)
                    # gather main/aux flats [16, nwin*512]
                    mainf = np.zeros((16, nwin * 512), dtype=np.float32)
                    auxf = np.zeros((16, (nwin + 2) * 512), dtype=np.float32)
                    for g, (gcol, nb) in enumerate(DUMP[lvl][pair]):
                        blk = o[:, gcol : gcol + 512 * nb].reshape(128, nb, 512)
                        for widx in range(2):
                            j = 2 * ci + widx
                            ws = [8 * g + 2 * b + widx for b in range(nb)]
                            ws = [w for w in ws if w < nwin]
                            for bi, w in enumerate(ws):
                                mainf[:, 512 * w : 512 * w + 512] = blk[32 * j : 32 * j + 16, bi]
                                auxf[:, 512 * w : 512 * w + 512] = blk[32 * j + 16 : 32 * j + 32, bi]
                    toks[:, c0 : c0 + ql] = mainf[:, :ql] + auxf[:, P : P + ql]
            piece = toks.reshape(CIN, s, P, P)[:, 0:n_i, 0:r, 0:r]
            dst = lvl_out_off[lvl] + i * s * r * r
            out[0, dst : dst + n_i * r * r] = (
                piece.transpose(1, 2, 3, 0).reshape(-1, CIN) + bs[lvl]
            )
    return out


def run(input, offsets, resolutions, weight, bias, trace=False, levels=None, **trace_kw):
    key = ("nc", tuple(levels) if levels is not None else None)
    if key not in _CACHE:
        _CACHE[key] = _build_program(levels)
    nc = _CACHE[key]
    in_maps = _pack_inputs(input, weight)
    res = run_bass_kernel_spmd(nc, in_maps, list(range(NCORES)), trace=trace, **trace_kw)
    return _unpack_outputs(res.results, bias, levels), res


def kernel(input, offsets, resolutions, weight, bias):
    out, _ = run(input, offsets, resolutions, weight, bias)
    return out


# revision 8
# speedup vs baseline: 1.8839x; 1.0055x over previous
"""Multi-level dense 3D conv (AbstractConv3D) as a Trainium2 Bass kernel.

v3: deep-contraction + dual-output-group + host realign.

Per level, the token axis is split into chunks processed in PAIRS on the
two 64-row PE strips (alternating strips keeps the ~34ns matmul cadence).
Each strip holds THREE z-shifted copies of the chunk's tokens (rows
16k+i = x[base + u + k], k=0..2), loaded by ONE overlapping-stride DMA
from HBM. A 512-token window is computed by 6 matmuls (48-deep, 32-wide
stationary): for each dx, matmul 'a' covers taps (dx,-1,dz) in the main
output group and (dx,0,dz) in the aux group (realigned by +P on the
host), matmul 'b' covers (dx,+1,dz) in main. Outputs are dumped raw
[128, OW] to DRAM; the host adds main+aux (and bias) during unpack.
"""

import math
from contextlib import ExitStack

import numpy as np
import ml_dtypes

import concourse.bass as bass
import concourse.bacc as bacc
import concourse.mybir as mybir
import concourse.tile as tile
from concourse.bass_utils import run_bass_kernel_spmd

BF16 = ml_dtypes.bfloat16

RES = [16, 18, 20, 23, 26, 29, 32, 36, 40, 45, 50, 56, 63, 70, 76, 80]
L = 16
CIN = 16
COUT = 16
NCORES = 8
NWIN = 512

# Per-level geometry
S_L = [math.ceil(r / 8) for r in RES]  # x-slabs per core
P_L = [r + 1 for r in RES]  # padded y/z extent
T_L = [(s + 2) * p * p for s, p in zip(S_L, P_L)]  # piece tokens (w/ x-halo)
TC_L = [t - 2 * p * p for t, p in zip(T_L, P_L)]  # computed tokens
GUARD = 1024
GAP = 1024  # zero gap between levels (reads stay within +-~700 of level)
T_IN = GUARD + sum(T_L) + (L - 1) * GAP + GUARD
LVL_IN_BASE = [GUARD + sum(T_L[:i]) + i * GAP for i in range(L)]

_CACHE = {}


def _tap_index(dx, dy, dz):
    return (dx + 1) * 9 + (dy + 1) * 3 + (dz + 1)


def _level_plan(lvl):
    """Chunking plan: nch chunks (even), qlen tokens each, nwin windows."""
    P = P_L[lvl]
    TC = TC_L[lvl]

    def geom(nch):
        qlen = math.ceil(TC / nch)
        nwin = (qlen - 1 + P) // NWIN + 1
        E = NWIN * (nwin - 1) + 2 * P * P + 2 * P + 529
        return qlen, nwin, E

    nch = 2
    while geom(nch)[2] > 40000:
        nch += 2
    qlen, nwin, E = geom(nch)
    return nch, qlen, nwin, E


PLAN = [_level_plan(l) for l in range(L)]


def _dump_layout():
    """Assign output-dump column ranges per (level, pair, psum-group).

    Group g covers windows w_abs=8g+w, w in [0, min(8, nwin-8g)).
    Slot of (chunk-strip ci, w): col strip j = w % 4, bank b = (w//4)*2 + ci.
    Dump col = groupbase + 512*b + u; rows 32j+(0:16 main | 16:32 aux).
    Returns per-level list of pair layouts and total dump width.
    """
    col = 0
    levels = []
    for lvl in range(L):
        nch, qlen, nwin, E = PLAN[lvl]
        pairs = []
        for p in range(nch // 2):
            groups = []
            w = 0
            while w < nwin:
                rem = min(8, nwin - w)
                nb = 2 * math.ceil(rem / 4)  # banks used this group
                groups.append((col, nb))
                col += 512 * nb
                w += 8
            pairs.append(groups)
        levels.append(pairs)
    return levels, col


DUMP, OW = _dump_layout()


def _build_program(levels=None):
    if levels is None:
        levels = range(L)
    nc = bacc.Bacc("TRN2", target_bir_lowering=False, debug=False, num_devices=NCORES)
    x_ext = nc.declare_dram_parameter("x", [16, T_IN], mybir.dt.bfloat16, isOutput=False)
    w_ext = nc.declare_dram_parameter("w", [128, L * 6 * 32], mybir.dt.bfloat16, isOutput=False)
    o_ext = nc.declare_dram_parameter("o", [128, OW], mybir.dt.bfloat16, isOutput=True)

    with tile.TileContext(nc) as tc, ExitStack() as ctx:
        w_pool = ctx.enter_context(tc.tile_pool(name="w", bufs=1))
        x_pool = ctx.enter_context(tc.tile_pool(name="x", bufs=2))
        ps_pool = ctx.enter_context(tc.tile_pool(name="ps", bufs=2, space="PSUM"))
        st_pool = ctx.enter_context(tc.tile_pool(name="st", bufs=2))

        w_sb = w_pool.tile([128, L * 6 * 32], mybir.dt.bfloat16)
        nc.sync.dma_start(w_sb[:], w_ext[:])

        for lvl in levels:
            _emit_level(nc, tc, x_pool, ps_pool, st_pool, w_sb, x_ext, o_ext, lvl)
    nc.finalize()
    return nc


def _emit_level(nc, tc, x_pool, ps_pool, st_pool, w_sb, x_ext, o_ext, lvl):
    P = P_L[lvl]
    nch, qlen, nwin, E = PLAN[lvl]
    PP = P * P

    for pair in range(nch // 2):
        xt = x_pool.tile([128, E], mybir.dt.bfloat16, tag="xchunk")
        # Load both chunks' 3 shifted copies: one overlapping-stride DMA each.
        for ci in range(2):
            c = 2 * pair + ci
            # chunk computed range starts at level token P*P + c*qlen
            B0 = LVL_IN_BASE[lvl] + PP + c * qlen - (PP + P + 2)
            SEG = 32768
            for s0 in range(0, E, SEG):
                sl = min(SEG, E - s0)
                for k in range(3):
                    nc.sync.dma_start(
                        xt[64 * ci + 16 * k : 64 * ci + 16 * k + 16, s0 : s0 + sl],
                        x_ext[:, B0 + k + s0 : B0 + k + s0 + sl],
                    )

        ngroups = math.ceil(nwin / 8)
        groups = DUMP[lvl][pair]
        g = 0
        while g < ngroups:
            # flush batch: up to 3 psum groups share one staging tile / out DMA
            gb = min(3, ngroups - g)
            bcols = sum(groups[g + k][1] * 512 for k in range(gb))
            st = st_pool.tile([128, 6144], mybir.dt.bfloat16, tag="stage")
            scol = 0
            for k in range(gb):
                gcol, nb = groups[g + k]
                ps = ps_pool.tile([128, 2048], mybir.dt.float32, tag="ps")
                for m in range(6):
                    kind, dxi = divmod(m, 3)
                    dx = dxi - 1
                    slot = lvl * 6 + kind * 3 + dxi
                    for w in range(min(8, nwin - 8 * (g + k))):
                        wa = 8 * (g + k) + w
                        if kind == 0:
                            off = NWIN * wa + dx * PP + PP + 1
                        else:
                            off = NWIN * wa + dx * PP + PP + 2 * P + 1
                        j = w % 4
                        for ci in range(2):
                            b = (w // 4) * 2 + ci
                            nc.tensor.matmul(
                                ps[32 * j : 32 * j + 32, 512 * b : 512 * b + 512],
                                w_sb[64 * ci : 64 * ci + 48, slot * 32 : slot * 32 + 32],
                                xt[64 * ci : 64 * ci + 48, off : off + 512],
                                start=(m == 0),
                                stop=(m == 5),
                                tile_position=(64 * ci, 32 * j),
                            )
                half = 256 * nb
                nc.scalar.copy(st[:, scol : scol + half], ps[:, 0:half])
                nc.vector.tensor_copy(
                    st[:, scol + half : scol + 512 * nb], ps[:, half : 512 * nb]
                )
                scol += 512 * nb
            nc.sync.dma_start(
                o_ext[:, groups[g][0] : groups[g][0] + bcols], st[:, 0:bcols]
            )
            g += gb


def _pack_inputs(input, weight):
    """Host-side pad/cast/transpose/shard. Returns per-core in_maps."""
    x = np.asarray(input)[0]  # [N, 16] f32
    wt = np.asarray(weight).reshape(L, 27, CIN, COUT)

    wb = np.zeros((128, L * 6 * 32), dtype=np.float32)
    for lvl in range(L):
        for kind in range(2):
            for dxi, dx in enumerate((-1, 0, 1)):
                slot = lvl * 6 + kind * 3 + dxi
                S = np.zeros((48, 32), dtype=np.float32)
                for k in range(3):
                    if kind == 0:
                        S[16 * k : 16 * k + 16, 0:16] = wt[lvl, _tap_index(dx, -1, k - 1)]
                        S[16 * k : 16 * k + 16, 16:32] = wt[lvl, _tap_index(dx, 0, k - 1)]
                    else:
                        S[16 * k : 16 * k + 16, 0:16] = wt[lvl, _tap_index(dx, 1, k - 1)]
                wb[0:48, slot * 32 : slot * 32 + 32] = S
                wb[64:112, slot * 32 : slot * 32 + 32] = S
    wb = wb.astype(BF16)

    xs = [np.zeros((16, T_IN), dtype=BF16) for _ in range(NCORES)]
    off = 0
    for lvl, r in enumerate(RES):
        P, s = P_L[lvl], S_L[lvl]
        g = x[off : off + r**3].reshape(r, r, r, CIN)
        off += r**3
        gp = np.zeros((CIN, 8 * s + 2, P, P), dtype=BF16)
        gp[:, 1 : r + 1, 0:r, 0:r] = g.transpose(3, 0, 1, 2)
        for i in range(NCORES):
            piece = gp[:, i * s : i * s + s + 2].reshape(CIN, T_L[lvl])
            xs[i][:, LVL_IN_BASE[lvl] : LVL_IN_BASE[lvl] + T_L[lvl]] = piece

    return [{"x": xs[i], "w": wb} for i in range(NCORES)]


def _unpack_outputs(results, bias, levels=None):
    """Assemble [1, N, 16] f32 from per-core raw dumps (main+aux+bias)."""
    if levels is None:
        levels = range(L)
    bs = np.asarray(bias, dtype=np.float32)
    n_total = sum(r**3 for r in RES)
    out = np.zeros((1, n_total, CIN), dtype=np.float32)
    lvl_out_off = np.concatenate([[0], np.cumsum([r**3 for r in RES])])
    for i in range(NCORES):
        o = np.asarray(results[i]["o"], dtype=np.float32)  # [128, OW]
        for lvl in levels:
            r = RES[lvl]
            P, s = P_L[lvl], S_L[lvl]
            nch, qlen, nwin, E = PLAN[lvl]
            TC = TC_L[lvl]
            n_i = min(s, r - i * s)
            if n_i <= 0:
                continue
            toks = np.zeros((16, TC), dtype=np.float32)
            for pair in range(nch // 2):
                for ci in range(2):
                    c = 2 * pair + ci
                    c0 = c * qlen
                    if c0 >= TC:
                        continue
                    ql = min(qlen, TC - c0)
                    # gather main/aux flats [16, nwin*512]
                    mainf = np.zeros((16, nwin * 512), dtype=np.float32)
                    auxf = np.zeros((16, (nwin + 2) * 512), dtype=np.float32)
                    for g, (gcol, nb) in enumerate(DUMP[lvl][pair]):
                        blk = o[:, gcol : gcol + 512 * nb].reshape(128, nb, 512)
                        for wl in range(min(8, nwin - 8 * g)):
                            w = 8 * g + wl
                            j = wl % 4
                            b = (wl // 4) * 2 + ci
                            mainf[:, 512 * w : 512 * w + 512] = blk[32 * j : 32 * j + 16, b]
                            auxf[:, 512 * w : 512 * w + 512] = blk[32 * j + 16 : 32 * j + 32, b]
                    toks[:, c0 : c0 + ql] = mainf[:, :ql] + auxf[:, P : P + ql]
            piece = toks.reshape(CIN, s, P, P)[:, 0:n_i, 0:r, 0:r]
            dst = lvl_out_off[lvl] + i * s * r * r
            out[0, dst : dst + n_i * r * r] = (
                piece.transpose(1, 2, 3, 0).reshape(-1, CIN) + bs[lvl]
            )
    return out


def run(input, offsets, resolutions, weight, bias, trace=False, levels=None, **trace_kw):
    key = ("nc", tuple(levels) if levels is not None else None)
    if key not in _CACHE:
        _CACHE[key] = _build_program(levels)
    nc = _CACHE[key]
    in_maps = _pack_inputs(input, weight)
    res = run_bass_kernel_spmd(nc, in_maps, list(range(NCORES)), trace=trace, **trace_kw)
    return _unpack_outputs(res.results, bias, levels), res


def kernel(input, offsets, resolutions, weight, bias):
    out, _ = run(input, offsets, resolutions, weight, bias)
    return out


# revision 13
# speedup vs baseline: 3.5518x; 1.8853x over previous
"""Multi-level dense 3D conv (AbstractConv3D) as a Trainium2 Bass kernel.

v3: deep-contraction + dual-output-group + host realign.

Per level, the token axis is split into chunks processed in PAIRS on the
two 64-row PE strips (alternating strips keeps the ~34ns matmul cadence).
Each strip holds THREE z-shifted copies of the chunk's tokens (rows
16k+i = x[base + u + k], k=0..2), loaded by ONE overlapping-stride DMA
from HBM. A 512-token window is computed by 6 matmuls (48-deep, 32-wide
stationary): for each dx, matmul 'a' covers taps (dx,-1,dz) in the main
output group and (dx,0,dz) in the aux group (realigned by +P on the
host), matmul 'b' covers (dx,+1,dz) in main. Outputs are dumped raw
[128, OW] to DRAM; the host adds main+aux (and bias) during unpack.
"""

import math
from contextlib import ExitStack

import numpy as np
import ml_dtypes

import concourse.bass as bass
import concourse.bacc as bacc
import concourse.mybir as mybir
import concourse.tile as tile
from concourse.bass_utils import run_bass_kernel_spmd

BF16 = ml_dtypes.bfloat16

RES = [16, 18, 20, 23, 26, 29, 32, 36, 40, 45, 50, 56, 63, 70, 76, 80]
L = 16
CIN = 16
COUT = 16
NCORES = 8
NWIN = 512

# Per-level geometry
S_L = [math.ceil(r / 8) for r in RES]  # x-slabs per core
P_L = [r + 1 for r in RES]  # padded y/z extent
T_L = [(s + 2) * p * p for s, p in zip(S_L, P_L)]  # piece tokens (w/ x-halo)
TC_L = [t - 2 * p * p for t, p in zip(T_L, P_L)]  # computed tokens
GUARD = 1024
GAP = 1024  # zero gap between levels (reads stay within +-~700 of level)
T_IN = GUARD + sum(T_L) + (L - 1) * GAP + GUARD
LVL_IN_BASE = [GUARD + sum(T_L[:i]) + i * GAP for i in range(L)]

_CACHE = {}


def _tap_index(dx, dy, dz):
    return (dx + 1) * 9 + (dy + 1) * 3 + (dz + 1)


def _level_plan(lvl):
    """Chunking plan: nch chunks (even), qlen tokens each, nwin windows."""
    P = P_L[lvl]
    TC = TC_L[lvl]

    def geom(nch):
        qlen = math.ceil(TC / nch)
        nwin = (qlen - 1 + P) // NWIN + 1
        E = NWIN * (nwin - 1) + 2 * P * P + 2 * P + 529
        return qlen, nwin, E

    nch = 2
    while geom(nch)[2] > 40000:
        nch += 2
    qlen, nwin, E = geom(nch)
    return nch, qlen, nwin, E


PLAN = [_level_plan(l) for l in range(L)]

# Fat input image: per chunk-pair one [128, E] block (rows 0:48 = chunk 2p's
# three z-shifted copies, rows 64:112 = chunk 2p+1's; rest zero), stored
# consecutively in DRAM so the kernel does full-width [128, W] loads.
FAT_BASE = []  # per (lvl, pair) column base
_fw = 0
for _l in range(L):
    _nch, _q, _nw, _E = PLAN[_l]
    FAT_BASE.append([])
    for _p in range(_nch // 2):
        FAT_BASE[_l].append(_fw)
        _fw += _E
FAT_W = _fw


def _dump_layout():
    """Assign output-dump column ranges per (level, pair, psum-group).

    Group g covers windows w_abs=8g+w, w in [0, min(8, nwin-8g)).
    Slot of (chunk-strip ci, w): col strip j = w % 4, bank b = (w//4)*2 + ci.
    Dump col = groupbase + 512*b + u; rows 32j+(0:16 main | 16:32 aux).
    Returns per-level list of pair layouts and total dump width.
    """
    col = 0
    levels = []
    for lvl in range(L):
        nch, qlen, nwin, E = PLAN[lvl]
        pairs = []
        for p in range(nch // 2):
            groups = []
            w = 0
            while w < nwin:
                rem = min(8, nwin - w)
                nb = 2 * math.ceil(rem / 4)  # banks used this group
                groups.append((col, nb))
                col += 512 * nb
                w += 8
            pairs.append(groups)
        levels.append(pairs)
    return levels, col


DUMP, OW = _dump_layout()


def _build_program(levels=None):
    if levels is None:
        levels = range(L)
    nc = bacc.Bacc("TRN2", target_bir_lowering=False, debug=False, num_devices=NCORES)
    x_ext = nc.declare_dram_parameter("x", [128, FAT_W], mybir.dt.bfloat16, isOutput=False)
    w_ext = nc.declare_dram_parameter("w", [128, L * 6 * 32], mybir.dt.bfloat16, isOutput=False)
    o_ext = nc.declare_dram_parameter("o", [128, OW], mybir.dt.bfloat16, isOutput=True)

    with tile.TileContext(nc) as tc, ExitStack() as ctx:
        w_pool = ctx.enter_context(tc.tile_pool(name="w", bufs=1))
        x_pool = ctx.enter_context(tc.tile_pool(name="x", bufs=2))
        ps_pool = ctx.enter_context(tc.tile_pool(name="ps", bufs=2, space="PSUM"))
        st_pool = ctx.enter_context(tc.tile_pool(name="st", bufs=2))

        w_sb = w_pool.tile([128, L * 6 * 32], mybir.dt.bfloat16)
        nc.sync.dma_start(w_sb[:], w_ext[:])

        for lvl in levels:
            _emit_level(nc, tc, x_pool, ps_pool, st_pool, w_sb, x_ext, o_ext, lvl)
    nc.finalize()
    return nc


def _emit_level(nc, tc, x_pool, ps_pool, st_pool, w_sb, x_ext, o_ext, lvl):
    P = P_L[lvl]
    nch, qlen, nwin, E = PLAN[lvl]
    PP = P * P

    for pair in range(nch // 2):
        xt = x_pool.tile([128, E], mybir.dt.bfloat16, tag="xchunk")
        # One fat [128, W] load per segment from the host-prepacked image.
        fb = FAT_BASE[lvl][pair]
        nseg = math.ceil(E / 28000)
        W = math.ceil(E / nseg)
        for s0 in range(0, E, W):
            sl = min(W, E - s0)
            nc.sync.dma_start(xt[:, s0 : s0 + sl], x_ext[:, fb + s0 : fb + s0 + sl])

        ngroups = math.ceil(nwin / 8)
        groups = DUMP[lvl][pair]
        g = 0
        while g < ngroups:
            # flush batch: up to 3 psum groups share one staging tile / out DMA
            gb = min(3, ngroups - g)
            bcols = sum(groups[g + k][1] * 512 for k in range(gb))
            st = st_pool.tile([128, 6144], mybir.dt.bfloat16, tag="stage")
            scol = 0
            for k in range(gb):
                gcol, nb = groups[g + k]
                ps = ps_pool.tile([128, 2048], mybir.dt.float32, tag="ps")
                rem = min(8, nwin - 8 * (g + k))
                for m in range(6):
                    kind, dxi = divmod(m, 3)
                    dx = dxi - 1
                    slot = lvl * 6 + kind * 3 + dxi
                    # stagger strip 1 by one window so consecutive matmuls
                    # differ in both row strip and column strip
                    for idx in range(rem):
                        for ci in range(2):
                            w = idx if ci == 0 else (idx + 1) % rem
                            wa = 8 * (g + k) + w
                            if kind == 0:
                                off = NWIN * wa + dx * PP + PP + 1
                            else:
                                off = NWIN * wa + dx * PP + PP + 2 * P + 1
                            j = w % 4
                            b = (w // 4) * 2 + ci
                            nc.tensor.matmul(
                                ps[32 * j : 32 * j + 32, 512 * b : 512 * b + 512],
                                w_sb[64 * ci : 64 * ci + 48, slot * 32 : slot * 32 + 32],
                                xt[64 * ci : 64 * ci + 48, off : off + 512],
                                start=(m == 0),
                                stop=(m == 5),
                                tile_position=(64 * ci, 32 * j),
                            )
                half = 256 * nb
                nc.scalar.copy(st[:, scol : scol + half], ps[:, 0:half])
                nc.vector.tensor_copy(
                    st[:, scol + half : scol + 512 * nb], ps[:, half : 512 * nb]
                )
                scol += 512 * nb
            nc.sync.dma_start(
                o_ext[:, groups[g][0] : groups[g][0] + bcols], st[:, 0:bcols]
            )
            g += gb


def _pack_inputs(input, weight):
    """Host-side pad/cast/transpose/shard. Returns per-core in_maps."""
    x = np.asarray(input)[0]  # [N, 16] f32
    wt = np.asarray(weight).reshape(L, 27, CIN, COUT)

    wb = np.zeros((128, L * 6 * 32), dtype=np.float32)
    for lvl in range(L):
        for kind in range(2):
            for dxi, dx in enumerate((-1, 0, 1)):
                slot = lvl * 6 + kind * 3 + dxi
                S = np.zeros((48, 32), dtype=np.float32)
                for k in range(3):
                    if kind == 0:
                        S[16 * k : 16 * k + 16, 0:16] = wt[lvl, _tap_index(dx, -1, k - 1)]
                        S[16 * k : 16 * k + 16, 16:32] = wt[lvl, _tap_index(dx, 0, k - 1)]
                    else:
                        S[16 * k : 16 * k + 16, 0:16] = wt[lvl, _tap_index(dx, 1, k - 1)]
                wb[0:48, slot * 32 : slot * 32 + 32] = S
                wb[64:112, slot * 32 : slot * 32 + 32] = S
    wb = wb.astype(BF16)

    xs = [np.zeros((16, T_IN), dtype=BF16) for _ in range(NCORES)]
    off = 0
    for lvl, r in enumerate(RES):
        P, s = P_L[lvl], S_L[lvl]
        g = x[off : off + r**3].reshape(r, r, r, CIN)
        off += r**3
        gp = np.zeros((CIN, 8 * s + 2, P, P), dtype=BF16)
        gp[:, 1 : r + 1, 0:r, 0:r] = g.transpose(3, 0, 1, 2)
        for i in range(NCORES):
            piece = gp[:, i * s : i * s + s + 2].reshape(CIN, T_L[lvl])
            xs[i][:, LVL_IN_BASE[lvl] : LVL_IN_BASE[lvl] + T_L[lvl]] = piece

    fats = []
    for i in range(NCORES):
        fat = np.zeros((128, FAT_W), dtype=BF16)
        for lvl in range(L):
            P = P_L[lvl]
            PP = P * P
            nch, qlen, nwin, E = PLAN[lvl]
            for pair in range(nch // 2):
                fb = FAT_BASE[lvl][pair]
                for ci in range(2):
                    c = 2 * pair + ci
                    B0 = LVL_IN_BASE[lvl] + PP + c * qlen - (PP + P + 2)
                    for k in range(3):
                        fat[64 * ci + 16 * k : 64 * ci + 16 * k + 16, fb : fb + E] = xs[
                            i
                        ][:, B0 + k : B0 + k + E]
        fats.append(fat)

    return [{"x": fats[i], "w": wb} for i in range(NCORES)]


def _unpack_outputs(results, bias, levels=None):
    """Assemble [1, N, 16] f32 from per-core raw dumps (main+aux+bias)."""
    if levels is None:
        levels = range(L)
    bs = np.asarray(bias, dtype=np.float32)
    n_total = sum(r**3 for r in RES)
    out = np.zeros((1, n_total, CIN), dtype=np.float32)
    lvl_out_off = np.concatenate([[0], np.cumsum([r**3 for r in RES])])
    for i in range(NCORES):
        o = np.asarray(results[i]["o"], dtype=np.float32)  # [128, OW]
        for lvl in levels:
            r = RES[lvl]
            P, s = P_L[lvl], S_L[lvl]
            nch, qlen, nwin, E = PLAN[lvl]
            TC = TC_L[lvl]
            n_i = min(s, r - i * s)
            if n_i <= 0:
                continue
            toks = np.zeros((16, TC), dtype=np.float32)
            for pair in range(nch // 2):
                for ci in range(2):
                    c = 2 * pair + ci
                    c0 = c * qlen
                    if c0 >= TC:
                        continue
                    ql = min(qlen, TC - c0)
                    # gather main/aux flats [16, nwin*512]
                    mainf = np.zeros((16, nwin * 512), dtype=np.float32)
                    auxf = np.zeros((16, (nwin + 2) * 512), dtype=np.float32)
                    for g, (gcol, nb) in enumerate(DUMP[lvl][pair]):
                        blk = o[:, gcol : gcol + 512 * nb].reshape(128, nb, 512)
                        for wl in range(min(8, nwin - 8 * g)):
                            w = 8 * g + wl
                            j = wl % 4
                            b = (wl // 4) * 2 + ci
                            mainf[:, 512 * w : 512 * w + 512] = blk[32 * j : 32 * j + 16, b]
                            auxf[:, 512 * w : 512 * w + 512] = blk[32 * j + 16 : 32 * j + 32, b]
                    toks[:, c0 : c0 + ql] = mainf[:, :ql] + auxf[:, P : P + ql]
            piece = toks.reshape(CIN, s, P, P)[:, 0:n_i, 0:r, 0:r]
            dst = lvl_out_off[lvl] + i * s * r * r
            out[0, dst : dst + n_i * r * r] = (
                piece.transpose(1, 2, 3, 0).reshape(-1, CIN) + bs[lvl]
            )
    return out


def run(input, offsets, resolutions, weight, bias, trace=False, levels=None, **trace_kw):
    key = ("nc", tuple(levels) if levels is not None else None)
    if key not in _CACHE:
        _CACHE[key] = _build_program(levels)
    nc = _CACHE[key]
    in_maps = _pack_inputs(input, weight)
    res = run_bass_kernel_spmd(nc, in_maps, list(range(NCORES)), trace=trace, **trace_kw)
    return _unpack_outputs(res.results, bias, levels), res


def kernel(input, offsets, resolutions, weight, bias):
    out, _ = run(input, offsets, resolutions, weight, bias)
    return out
